# revision 13
# baseline (speedup 1.0000x reference)
"""Fused co-memory cross-attention kernel for Trainium2, SPMD over 8 NeuronCores.

Module: LayerNorm(q/k/v) -> per-head projections -> masked softmax attention
        -> output projection.  B=2, Sq=1024, Sk=5*1024, C=256, 8 heads x 32.

v3 sharding: (batch x head-pair) parallel = 2 x 4 cores.  Each core handles 2
of the 8 heads for ALL 1024 queries of its batch against the batch's
mask-compacted keys; the output projection is computed per-core over its two
heads' rows of Wo and the four partial outputs of a batch are summed on the
host (free).  This removes the 4x duplication of the K/V projections that the
query-split sharding had, pushing PE work below the exp roofline.

Per-core pipeline (all on-chip; ScalarE does nothing but the 64 exp calls,
which pace the kernel at ~1.15us each):
  - K/V arrive host-transposed (kT/vT [C, Sk] fp16), loaded in per-chunk DMAs
    spread over the sync/scalar/gpsimd rings.
  - token means via M=1 matmuls (lhsT = -ones/C, LDW P=1), cast rows bounced
    through DRAM into partition-broadcast tiles; kT/vT centered in place on
    GpSimd; variance via squared tiles + M=tok matmuls; rstd via quake rsqrt.
  - K's rstd rides the exp scale port; V's rstd is folded into the v-proj
    drain; the bm@Wk score bias is dropped (softmax-invariant).
  - vh carries a ones column -> PV (M=33) accumulates the softmax denominator
    as ctx row 32.
  - scores: K=32 matmuls, row groups (0,0)/(32,0) into separate PSUM banks
    (concurrent pairs); PV N=512 accumulating matmuls into 4 ctx banks.
  - normalize via PE broadcast of 1/den; partial out-proj (64 rows of Wo).
"""

import math
import os

import numpy as np

HEADS = 8
KD = 32
C = 256
EPS = 1e-3
B = 2
SQ = 1024
FTOK = 1024
TK = 5
NCORES = 8
NPAIR = 4                  # head pairs per batch; cores = B * NPAIR
NEG = -1.0e9
P = 128

_cache: dict = {}

last_exec_time_ns = None
last_results = None


def _build_program(F: int, fp16: bool = True):
    from contextlib import ExitStack

    import concourse.bass as bass
    import concourse.tile as tile
    from concourse import bacc, mybir

    dt = mybir.dt
    f32 = dt.float32
    mdt = dt.float16 if fp16 else dt.float32
    AF = mybir.ActivationFunctionType
    OP = mybir.AluOpType
    i32 = dt.int32
    SK = F * FTOK
    NT = SK // P
    NCH = NT // 4

    nc = bacc.Bacc("TRN2", target_bir_lowering=False, debug=False,
                   num_devices=NCORES)

    xq_d = nc.dram_tensor("xq", [SQ, C], mdt, kind="ExternalInput").ap()
    kT_d = nc.dram_tensor("kT", [2, P, SK], mdt, kind="ExternalInput").ap()
    vT_d = nc.dram_tensor("vT", [2, P, SK], mdt, kind="ExternalInput").ap()
    wq_d = nc.dram_tensor("wq", [C, 64], mdt, kind="ExternalInput").ap()
    wk_d = nc.dram_tensor("wk", [C, 64], mdt, kind="ExternalInput").ap()
    wv_d = nc.dram_tensor("wv", [C, 64], mdt, kind="ExternalInput").ap()
    wo_d = nc.dram_tensor("wo", [64, C], mdt, kind="ExternalInput").ap()
    tq_d = nc.dram_tensor("tq", [64, 1], f32, kind="ExternalInput").ap()
    fb_d = nc.dram_tensor("fbias", [1, F], f32, kind="ExternalInput").ap()
    out_d = nc.dram_tensor("out", [SQ, C], f32, kind="ExternalOutput").ap()
    mu_d = nc.dram_tensor("mu_scr", [2, SK], mdt).ap()
    qt_d = nc.dram_tensor("qt_scr", [SQ, C], mdt).ap()

    with tile.TileContext(nc) as tc, ExitStack() as ctx:
        singles = ctx.enter_context(tc.tile_pool(name="singles", bufs=1))
        io_p = ctx.enter_context(tc.tile_pool(name="io", bufs=4))
        sq_p = ctx.enter_context(tc.tile_pool(name="sqp", bufs=3))
        exp_p = ctx.enter_context(tc.tile_pool(name="exp", bufs=4))
        out_p = ctx.enter_context(tc.tile_pool(name="outp", bufs=2))
        stats_p = ctx.enter_context(tc.tile_pool(name="stats", bufs=4))
        ps_ctx = ctx.enter_context(
            tc.tile_pool(name="ps_ctx", bufs=1, space="PSUM"))
        ps_sc = ctx.enter_context(
            tc.tile_pool(name="ps_sc", bufs=2, space="PSUM"))

        dmae = [nc.sync, nc.scalar, nc.gpsimd]

        def sct(name):
            return ps_sc.tile([P, 1024], f32, tag="sc", name=name)

        # ---- constants / weights ----
        negmc = singles.tile([P, 1], mdt)
        nc.vector.memset(negmc[:], -1.0 / C)
        posc = singles.tile([P, 1], mdt)
        nc.vector.memset(posc[:], 1.0 / C)
        ones32 = singles.tile([1, 32], mdt)
        nc.vector.memset(ones32[:], 1.0)
        fb_t = singles.tile([P, F], f32)
        nc.sync.dma_start(
            out=fb_t[:],
            in_=bass.AP(tensor=fb_d.tensor, offset=fb_d.offset,
                        ap=[[0, P], [1, F]]))
        tq_t = singles.tile([64, 1], f32, tag="tq")
        nc.sync.dma_start(out=tq_t[:], in_=tq_d)

        w_tiles = {}
        for name, d in (("wq", wq_d), ("wk", wk_d), ("wv", wv_d)):
            for kt in range(2):
                t = singles.tile([P, 64], mdt, tag=f"{name}{kt}")
                nc.sync.dma_start(out=t[:], in_=d[kt * P:(kt + 1) * P, :])
                w_tiles[(name, kt)] = t
        wo_t = singles.tile([64, C], mdt, tag="wo")
        nc.sync.dma_start(out=wo_t[:], in_=wo_d)

        # K/V loads: per-chunk DMAs spread across rings
        kTt = [singles.tile([P, SK], mdt, tag=f"kT{h}", name=f"kT{h}")
               for h in range(2)]
        vTt = [singles.tile([P, SK], mdt, tag=f"vT{h}", name=f"vT{h}")
               for h in range(2)]
        di = 0
        for ch in range(NCH):
            cs = ch * 4 * P
            for h in range(2):
                dmae[di % 2].dma_start(out=kTt[h][:, cs:cs + 4 * P],
                                       in_=kT_d[h, :, cs:cs + 4 * P])
                di += 1
                dmae[di % 2].dma_start(out=vTt[h][:, cs:cs + 4 * P],
                                       in_=vT_d[h, :, cs:cs + 4 * P])
                di += 1

        kp = singles.tile([64, SK], mdt, tag="kp", name="kp")
        vh = singles.tile([P, NT, 2, 33], mdt, tag="vh")
        nc.vector.memset(vh[:], 1.0)
        rstd_k = singles.tile([P, NT], f32, tag="rstd_k")
        rstd_v = singles.tile([P, NT], f32, tag="rstd_v")
        mub = [singles.tile([P, SK], mdt, tag=f"mub{i}", name=f"mub{i}")
               for i in range(2)]
        murow = singles.tile([1, 2, SK], mdt, tag="murow")

        def quake_rsqrt(dst_ap, var_ap, n):
            ve = stats_p.tile([P, 8], f32, tag="ve", name="ve")
            nc.vector.tensor_scalar(ve[:, 0:n], var_ap, scalar1=EPS,
                                    scalar2=None, op0=OP.add)
            y = stats_p.tile([P, 8], f32, tag="y0", name="y0")
            nc.vector.tensor_scalar(y[:, 0:n].bitcast(i32),
                                    ve[:, 0:n].bitcast(i32),
                                    scalar1=1, scalar2=None,
                                    op0=OP.logical_shift_right)
            nc.vector.tensor_scalar(y[:, 0:n].bitcast(i32),
                                    y[:, 0:n].bitcast(i32),
                                    scalar1=-1, scalar2=0x5F3759DF,
                                    op0=OP.mult, op1=OP.add)
            c1 = stats_p.tile([P, 8], f32, tag="nc", name="nwt")
            nc.vector.tensor_mul(c1[:, 0:n], y[:, 0:n], y[:, 0:n])
            nc.vector.tensor_mul(c1[:, 0:n], c1[:, 0:n], ve[:, 0:n])
            nc.vector.tensor_scalar(c1[:, 0:n], c1[:, 0:n], scalar1=-0.5,
                                    scalar2=1.5, op0=OP.mult, op1=OP.add)
            y2 = stats_p.tile([P, 8], f32, tag="yn", name="yn")
            nc.vector.tensor_mul(y2[:, 0:n], y[:, 0:n], c1[:, 0:n])
            c2 = stats_p.tile([P, 8], f32, tag="nc2", name="nwt2")
            nc.vector.tensor_mul(c2[:, 0:n], y2[:, 0:n], y2[:, 0:n])
            nc.vector.tensor_mul(c2[:, 0:n], c2[:, 0:n], ve[:, 0:n])
            nc.vector.tensor_scalar(c2[:, 0:n], c2[:, 0:n], scalar1=-0.5,
                                    scalar2=1.5, op0=OP.mult, op1=OP.add)
            nc.vector.tensor_mul(dst_ap, y2[:, 0:n], c2[:, 0:n])

        # ---- emission pieces ----
        def emit_means(ch):
            cs = ch * 4 * P
            mean_ps = sct("mean_ps")
            for half in range(2):
                nc.tensor.matmul(mean_ps[0:1, 0:512],
                                 negmc[:], kTt[half][:, cs:cs + 4 * P],
                                 start=(half == 0), stop=(half == 1),
                                 skip_group_check=True)
            for half in range(2):
                nc.tensor.matmul(mean_ps[32:33, 0:512],
                                 negmc[:], vTt[half][:, cs:cs + 4 * P],
                                 start=(half == 0), stop=(half == 1),
                                 skip_group_check=True)
            nc.vector.tensor_copy(murow[:, 0, cs:cs + 4 * P],
                                  mean_ps[0:1, 0:512])
            nc.vector.tensor_copy(murow[:, 1, cs:cs + 4 * P],
                                  mean_ps[32:33, 0:512])
            nc.sync.dma_start(out=mu_d[0:1, cs:cs + 4 * P],
                              in_=murow[:, 0, cs:cs + 4 * P])
            nc.sync.dma_start(out=mu_d[1:2, cs:cs + 4 * P],
                              in_=murow[:, 1, cs:cs + 4 * P])
            for i in range(2):
                nc.gpsimd.dma_start(
                    out=mub[i][:, cs:cs + 4 * P],
                    in_=mu_d[i:i + 1, cs:cs + 4 * P].partition_broadcast(P))

        def emit_center(ch):
            cs = ch * 4 * P
            for half in range(2):
                nc.vector.tensor_tensor(
                    kTt[half][:, cs:cs + 4 * P], kTt[half][:, cs:cs + 4 * P],
                    mub[0][:, cs:cs + 4 * P], op=OP.add)
                nc.vector.tensor_tensor(
                    vTt[half][:, cs:cs + 4 * P], vTt[half][:, cs:cs + 4 * P],
                    mub[1][:, cs:cs + 4 * P], op=OP.add)

        def emit_kp(ch):
            cs = ch * 4 * P
            kps = sct("kp_ps")
            for kt in range(2):
                nc.tensor.matmul(
                    kps[0:64, 0:512],
                    w_tiles[("wk", kt)][:],
                    kTt[kt][:, cs:cs + 4 * P],
                    start=(kt == 0), stop=(kt == 1),
                    skip_group_check=True)
            nc.vector.tensor_copy(kp[:, cs:cs + 4 * P], kps[0:64, 0:512])

        def emit_sq(ch):
            cs = ch * 4 * P
            sqk = sq_p.tile([P, 512], mdt, tag="sqk", name="sqk")
            sqv = sq_p.tile([P, 512], mdt, tag="sqv", name="sqv")
            sqt = sq_p.tile([P, 512], mdt, tag="sqt", name="sqt")
            nc.vector.tensor_mul(sqk[:], kTt[0][:, cs:cs + 4 * P],
                                 kTt[0][:, cs:cs + 4 * P])
            nc.vector.tensor_mul(sqt[:], kTt[1][:, cs:cs + 4 * P],
                                 kTt[1][:, cs:cs + 4 * P])
            nc.vector.tensor_tensor(sqk[:], sqk[:], sqt[:], op=OP.add)
            nc.vector.tensor_mul(sqv[:], vTt[0][:, cs:cs + 4 * P],
                                 vTt[0][:, cs:cs + 4 * P])
            nc.vector.tensor_mul(sqt[:], vTt[1][:, cs:cs + 4 * P],
                                 vTt[1][:, cs:cs + 4 * P])
            nc.vector.tensor_tensor(sqv[:], sqv[:], sqt[:], op=OP.add)
            sqs = sct("sq_ps")
            for j in range(4):
                nc.tensor.matmul(sqs[:, j:j + 1],
                                 sqk[:, j * P:(j + 1) * P], posc[:],
                                 start=True, stop=True, skip_group_check=True)
                nc.tensor.matmul(sqs[:, 4 + j:5 + j],
                                 sqv[:, j * P:(j + 1) * P], posc[:],
                                 start=True, stop=True, skip_group_check=True)
            var8 = stats_p.tile([P, 8], f32, tag="var8", name="var8")
            nc.vector.tensor_copy(var8[:], sqs[:, 0:8])
            quake_rsqrt(rstd_k[:, 4 * ch:4 * ch + 4], var8[:, 0:4], 4)
            quake_rsqrt(rstd_v[:, 4 * ch:4 * ch + 4], var8[:, 4:8], 4)

        def emit_v(ch, j):
            t = 4 * ch + j
            ts = t * P
            vps = sct("v_ps")
            for kt in range(2):
                nc.tensor.matmul(
                    vps[:, 0:64],
                    vTt[kt][:, ts:ts + P],
                    w_tiles[("wv", kt)][:],
                    start=(kt == 0), stop=(kt == 1),
                    skip_group_check=True)
            nc.vector.tensor_scalar(
                vh[:, t, :, 0:32],
                bass.AP(tensor=vps.tensor, offset=vps.offset,
                        ap=[[1024, P], [32, 2], [1, 32]]),
                scalar1=rstd_v[:, t:t + 1], scalar2=None, op0=OP.mult)

        # ---- Q path ----
        xq_sb = singles.tile([P, 8, C], mdt, tag="xq_sb", name="xq_sb")
        for qt in range(8):
            dmae[qt % 2].dma_start(out=xq_sb[:, qt, :],
                                   in_=xq_d[qt * P:(qt + 1) * P, :])
        xts = [xq_sb[:, qt, :] for qt in range(8)]
        mvb = stats_p.tile([P, 8, 2], f32, tag="mv", name="mvb")
        for i, x_t in enumerate(xts):
            st = stats_p.tile([P, 6], f32, tag="bn", name="st")
            nc.vector.bn_stats(st[:], x_t)
            nc.vector.bn_aggr(mvb[:, i, :], st[:])
        rstdq = stats_p.tile([P, 8], f32, tag="rstdq", name="rstdq")
        quake_rsqrt(rstdq[:, 0:8], mvb[:, :, 1], 8)
        xqT = singles.tile([P, 2, SQ], mdt, tag="xqT")
        for qt in range(8):
            nmr = stats_p.tile([P, 1], f32, tag="nmr", name="nmr")
            nc.vector.tensor_scalar(nmr[:], mvb[:, qt, 0:1],
                                    scalar1=rstdq[:, qt:qt + 1],
                                    scalar2=-1.0, op0=OP.mult, op1=OP.mult)
            xh = io_p.tile([P, C], mdt, tag="xh", name="xh")
            nc.vector.tensor_scalar(xh[:], xts[qt],
                                    scalar1=rstdq[:, qt:qt + 1],
                                    scalar2=nmr[:], op0=OP.mult, op1=OP.add)
            nc.sync.dma_start(out=qt_d[qt * P:(qt + 1) * P, :], in_=xh[:])
        for ct in range(2):
            nc.sync.dma_start_transpose(
                out=xqT[:, ct, :], in_=qt_d[:, ct * P:(ct + 1) * P])

        qp = singles.tile([64, SQ], mdt, tag="qp", name="qp")

        def emit_qproj(qh):
            qps = sct("q_ps")
            for kt in range(2):
                nc.tensor.matmul(
                    qps[0:64, 0:512],
                    w_tiles[("wq", kt)][:],
                    xqT[:, kt, qh * 512:(qh + 1) * 512],
                    start=(kt == 0), stop=(kt == 1),
                    skip_group_check=True)
            nc.vector.tensor_scalar(
                qp[:, qh * 512:(qh + 1) * 512], qps[0:64, 0:512],
                scalar1=tq_t[:, 0:1], scalar2=None, op0=OP.add)

        # ---- attention ----
        ctx_ps = [ps_ctx.tile([P, 512], f32, tag=f"ctx{i}", name=f"ctx{i}")
                  for i in range(4)]

        def emit_scores(t, qh):
            ts = t * P
            f = t // (FTOK // P)
            sc = sct("sc")
            for e in range(2):
                nc.tensor.matmul(
                    sc[:, e * 512:(e + 1) * 512],
                    kp[32 * e:32 * e + 32, ts:ts + P],
                    qp[32 * e:32 * e + 32, qh * 512:(qh + 1) * 512],
                    start=True, stop=True, tile_position=(32 * e, 0),
                    skip_group_check=True)
            ex = exp_p.tile([P, 1024], mdt, tag="exp")
            nc.scalar.activation(ex[:], sc[:], AF.Exp,
                                 bias=fb_t[:, f:f + 1],
                                 scale=rstd_k[:, t:t + 1])
            return ex

        def emit_pv(t, qh, ex):
            for e in range(2):
                i = 2 * e + qh
                base = 64 * (i % 2)
                nc.tensor.matmul(
                    ctx_ps[i][base:base + 33, 0:512],
                    vh[:, t, e, 0:33],
                    ex[:, e * 512:(e + 1) * 512],
                    start=(t == 0), stop=(t == NT - 1),
                    tile_position=(0, base),
                    skip_group_check=True)

        # ---- pipelined emission ----
        pending = None
        for r in range(-2, NCH):
            pieces = []
            if r + 2 < NCH:
                pieces.append(lambda c=r + 2: emit_means(c))
            if r == -2:
                pieces.append(lambda: emit_qproj(0))
                pieces.append(lambda: emit_qproj(1))
            if 0 <= r + 1 < NCH:
                pieces.append(lambda c=r + 1: emit_center(c))
                pieces.append(lambda c=r + 1: emit_kp(c))
                pieces.append(lambda c=r + 1: emit_sq(c))
                for j in range(4):
                    pieces.append(lambda c=r + 1, jj=j: emit_v(c, jj))
            slots = ([(t, qh) for t in range(4 * r, 4 * r + 4)
                      for qh in range(2)] if r >= 0 else [])
            n = max(len(pieces), len(slots))
            for i in range(n):
                if i < len(pieces):
                    pieces[i]()
                if i < len(slots):
                    t, qh = slots[i]
                    ex = emit_scores(t, qh)
                    if pending is not None:
                        emit_pv(*pending)
                    pending = (t, qh, ex)
        if pending is not None:
            emit_pv(*pending)

        # ---- normalize + partial output projection ----
        ctxn = singles.tile([64, SQ], mdt, tag="ctxn", name="ctxn")
        for e in range(2):
            for qh in range(2):
                i = 2 * e + qh
                base = 64 * (i % 2)
                bank = ctx_ps[i]
                rden = out_p.tile([1, 512], mdt, tag="rden", name="rden")
                with nc.allow_low_precision(reason="1/den f16 data path"):
                    nc.vector.reciprocal(rden[:],
                                         bank[base + 32:base + 33, 0:512])
                rps = sct("rden_ps")
                nc.tensor.matmul(rps[0:32, 0:512], ones32[:], rden[:],
                                 start=True, stop=True, skip_group_check=True)
                rbc = out_p.tile([32, 512], mdt, tag="rbc", name="rbc")
                nc.vector.tensor_copy(rbc[:], rps[0:32, 0:512])
                nc.vector.tensor_tensor(
                    ctxn[32 * e:32 * e + 32, qh * 512:(qh + 1) * 512],
                    bank[base:base + 32, 0:512], rbc[:], op=OP.mult)

        for qt in range(8):
            ops = sct("o_ps")
            nc.tensor.matmul(
                ops[:, 0:256],
                ctxn[:, qt * P:(qt + 1) * P],
                wo_t[:],
                start=True, stop=True, skip_group_check=True)
            ot = out_p.tile([P, C], f32, tag="ot")
            nc.vector.tensor_copy(ot[:], ops[:, 0:256])
            dmae[qt % 2].dma_start(out=out_d[qt * P:(qt + 1) * P, :],
                                   in_=ot[:])

    nc.compile()
    return nc


def _get_program(F: int, use_tk: bool = False, fp16: bool = True):
    key = (F, fp16)
    if key not in _cache:
        _cache[key] = _build_program(F, fp16)
    return _cache[key]


def _prep_host(encoder_output, memory_key, memory_value, Wq, Wk, Wv, Wo,
               gamma_q, beta_q, gamma_m, beta_m, memory_mask, fp16=True):
    f32 = np.float32
    mdt = np.float16 if fp16 else np.float32
    enc = np.ascontiguousarray(
        np.asarray(encoder_output, dtype=f32).reshape(B, SQ, C))
    mk = np.asarray(memory_key, dtype=f32).reshape(B, TK, FTOK, C)
    mv = np.asarray(memory_value, dtype=f32).reshape(B, TK, FTOK, C)
    mask = np.asarray(memory_mask).astype(np.int64)

    gq = np.asarray(gamma_q, dtype=f32)
    bq = np.asarray(beta_q, dtype=f32)
    gm = np.asarray(gamma_m, dtype=f32)
    bm = np.asarray(beta_m, dtype=f32)
    Wq = np.asarray(Wq, dtype=f32)
    Wk = np.asarray(Wk, dtype=f32)
    Wv = np.asarray(Wv, dtype=f32)
    Wo = np.ascontiguousarray(np.asarray(Wo, dtype=f32))

    s = 1.0 / math.sqrt(KD)
    wq2 = np.ascontiguousarray(gq[:, None] * Wq * s)
    tq = (bq @ Wq * s).reshape(C)
    wk2 = np.ascontiguousarray(gm[:, None] * Wk)
    wv2 = np.ascontiguousarray(gm[:, None] * Wv)
    tv = (bm @ Wv).reshape(C)
    # bm @ Wk score bias is softmax-invariant -> dropped.
    # tv is applied on the host: out += (tv @ Wo).

    sel = []
    counts = []
    for b in range(B):
        act = np.nonzero(mask[b])[0]
        if len(act) == 0:
            sel.append((list(range(TK)), True))
            counts.append(TK)
        else:
            sel.append((list(act), False))
            counts.append(len(act))
    F = max(counts)

    per_batch = []
    for b in range(B):
        frames, uniform = sel[b]
        fb = np.zeros((1, F), dtype=f32)
        fr = list(frames)
        while len(fr) < F:
            fr.append(frames[-1])
            fb[0, len(fr) - 1] = NEG
        kb = mk[b][fr].reshape(F * FTOK, C)
        vb = mv[b][fr].reshape(F * FTOK, C)
        kbT = np.ascontiguousarray(kb.T.reshape(2, P, F * FTOK))
        vbT = np.ascontiguousarray(vb.T.reshape(2, P, F * FTOK))
        per_batch.append(dict(kT=kbT.astype(mdt), vT=vbT.astype(mdt),
                              fbias=fb, uniform=uniform))

    in_maps = []
    for c in range(NCORES):
        b = c // NPAIR
        p = c % NPAIR
        pb = per_batch[b]
        hs = slice(64 * p, 64 * p + 64)
        if pb["uniform"]:
            wq_c = np.zeros((C, 64), dtype=f32)
            tq_c = np.zeros((64, 1), dtype=f32)
        else:
            wq_c = wq2[:, hs]
            tq_c = tq[hs].reshape(64, 1)
        m = dict(
            kT=pb["kT"], vT=pb["vT"], fbias=pb["fbias"],
            xq=np.ascontiguousarray(enc[b]).astype(mdt),
            wq=np.ascontiguousarray(wq_c).astype(mdt),
            tq=np.ascontiguousarray(tq_c),
            wk=np.ascontiguousarray(wk2[:, hs]).astype(mdt),
            wv=np.ascontiguousarray(wv2[:, hs]).astype(mdt),
            wo=np.ascontiguousarray(Wo[hs, :]).astype(mdt),
        )
        in_maps.append(m)
    return F, (tv @ Wo), in_maps


def kernel(encoder_output, memory_key, memory_value, Wq, Wk, Wv, Wo,
           gamma_q, beta_q, gamma_m, beta_m, memory_mask):
    global last_exec_time_ns, last_results
    from concourse.bass_utils import run_bass_kernel_spmd

    fp16 = os.environ.get("KERNEL_FP32", "0") != "1"
    F, tvo, in_maps = _prep_host(
        encoder_output, memory_key, memory_value, Wq, Wk, Wv, Wo,
        gamma_q, beta_q, gamma_m, beta_m, memory_mask, fp16=fp16)
    nc = _get_program(F, False, fp16)

    trace = os.environ.get("BASS_KERNEL_TRACE", "0") == "1"
    res = run_bass_kernel_spmd(nc, in_maps, core_ids=list(range(NCORES)),
                               trace=trace)
    last_exec_time_ns = res.exec_time_ns
    last_results = res

    out = np.zeros((B, SQ, C), dtype=np.float32)
    for c in range(NCORES):
        b = c // NPAIR
        out[b] += res.results[c]["out"]
    out += tvo[None, None, :]
    return out.reshape(B, 1, 32, 32, C)
